# revision 40
# baseline (speedup 1.0000x reference)
"""CSAEncoder Trainium2 kernel: 3-branch cross-attention + concat DoubleConv.

Sharding (8 cores): 2 batch groups x 4 tensor ranks.
Core c: batch b = c // 4, rank g = c % 4.
  - Attention: core computes heads [4g, 4g+4) of all 3 branches for batch b
    (a contiguous 128-channel slab of each branch's output).
  - conv1 computed as partial sums over the core's local 384 input channels
    for ALL 512 output channels; one bf16 AllReduce(add) within the 4-core
    batch group gives every rank the full conv1 pre-BN output.
  - conv2 computed locally: full 512-channel contraction, only the core's own
    128 output channels. No second collective.
Host assembles the full (2, 512, 32, 32) output from the 8 per-core slabs.

v2 schedule: the attention is emitted as a flat pipeline of 12 head-slots
(branch x head). Each slot emits its 16 score matmuls unit-by-unit with the
previous slot's PV matmuls and filler work (projections for later branches,
conv1 blocks of finished branches) interleaved, so the in-order PE queue never
blocks on the scalar engine's exp stream and HAM stays at K=8/8. The softmax
denominator is produced broadcast on PSUM partitions 32:64 via 32 ones-columns
in the PV stationary, so normalization is a single DVE divide (no reciprocal,
no DRAM round-trip). conv1 partials accumulate in bf16 and are AllReduced in
bf16 as one chunk, with a tiny warmup collective issued at kernel start to pay
the first-call collective cost off the critical path.
"""

import os
import sys

import ml_dtypes
import numpy as np

for _p in ("/opt/trn_rl_repo",):
    if _p not in sys.path and os.path.isdir(_p):
        sys.path.insert(0, _p)

import concourse.bass as bass
import concourse.mybir as mybir
import concourse.tile as tile
from concourse import bacc
from concourse.bass_utils import run_bass_kernel_spmd

F32 = mybir.dt.float32
BF16 = mybir.dt.bfloat16
AF = mybir.ActivationFunctionType
ALU = mybir.AluOpType

B, C, H, W, HEADS = 2, 512, 32, 32, 16
D = C // HEADS            # 32
S = H * W                 # 1024
EPS = 1e-5
ISQD = 1.0 / np.sqrt(D)   # folded into the exp activation
NCORES = 8
GROUPS = [[0, 1, 2, 3], [4, 5, 6, 7]]
HP = W + 2                # padded row stride (34)


def build_nc():
    nc = bacc.Bacc(None, target_bir_lowering=False)

    # ---- per-core external inputs -------------------------------------
    x4_d = nc.declare_dram_parameter("x4", [4, 128, S], BF16, isOutput=False)
    oth_d = nc.declare_dram_parameter("oth", [2, 4, 128, S], BF16, isOutput=False)
    wqT_d = nc.declare_dram_parameter("wqT", [3, 4, 128, 128], BF16, isOutput=False)
    wkT_d = nc.declare_dram_parameter("wkT", [3, 4, 128, 128], BF16, isOutput=False)
    wvoT_d = nc.declare_dram_parameter("wvoT", [4, 128, 384], BF16, isOutput=False)
    dvec_d = nc.declare_dram_parameter("dvec", [128, 9], F32, isOutput=False)
    wobv_d = nc.declare_dram_parameter("wobv", [1, 384], F32, isOutput=False)
    c1wT_d = nc.declare_dram_parameter("c1wT", [3, 4, 128, 9, 128], BF16, isOutput=False)
    c2wT_d = nc.declare_dram_parameter("c2wT", [4, 128, 9, 128], BF16, isOutput=False)
    avec_d = nc.declare_dram_parameter("avec", [128, 10], F32, isOutput=False)
    out_d = nc.declare_dram_parameter("out", [128, S], F32, isOutput=True)

    with tile.TileContext(nc) as tc:
        import contextlib
        from collections import deque

        ctx = contextlib.ExitStack()
        with ctx:
            const = ctx.enter_context(tc.tile_pool(name="const", bufs=1))
            kq = ctx.enter_context(tc.tile_pool(name="kq", bufs=1))
            xtp = ctx.enter_context(tc.tile_pool(name="xtp", bufs=1))
            stg = ctx.enter_context(tc.tile_pool(name="stg", bufs=4))
            scps = ctx.enter_context(tc.tile_pool(name="scps", bufs=2, space="PSUM"))
            smps = ctx.enter_context(tc.tile_pool(name="smps", bufs=4, space="PSUM"))
            dram = ctx.enter_context(tc.tile_pool(name="dram", bufs=1, space="DRAM"))

            # ---- activations first (highest DMA priority) ----------------
            x_sb = const.tile([128, 4, S], BF16)
            oth_sb = const.tile([128, 2, 4, S], BF16)
            wq_sb = const.tile([128, 3, 4, 128], BF16)
            wk_sb = const.tile([128, 3, 4, 128], BF16)
            wvo_sb = const.tile([128, 4, 384], BF16)
            # DMA order = need order: x + branch-0 weights + q-source (oth0)
            # gate the first scores; everything else lands under branch 0.
            for ks in range(4):
                nc.sync.dma_start(out=x_sb[:, ks, :], in_=x4_d[ks])
            for ks in range(4):
                nc.sync.dma_start(out=wk_sb[:, 0, ks, :], in_=wkT_d[0, ks])
            for ks in range(4):
                nc.sync.dma_start(out=oth_sb[:, 0, ks, :], in_=oth_d[0, ks])
            for ks in range(4):
                nc.sync.dma_start(out=wq_sb[:, 0, ks, :], in_=wqT_d[0, ks])
            for ks in range(4):
                nc.sync.dma_start(out=wvo_sb[:, ks, :], in_=wvoT_d[ks])
            for i in (1, 2):
                for ks in range(4):
                    nc.sync.dma_start(out=wk_sb[:, i, ks, :], in_=wkT_d[i, ks])
                    nc.sync.dma_start(out=wq_sb[:, i, ks, :], in_=wqT_d[i, ks])
            for ks in range(4):
                nc.sync.dma_start(out=oth_sb[:, 1, ks, :], in_=oth_d[1, ks])

            # Small consts: DMA to staging, then re-own on the consuming
            # engine (DVE / ACT) so consumers need no cross-engine const wait.
            dvec_st = const.tile([128, 9], F32)
            nc.gpsimd.dma_start(out=dvec_st, in_=dvec_d[:])
            wobv_st = const.tile([128, 384], F32)
            nc.gpsimd.dma_start(out=wobv_st, in_=wobv_d[:].partition_broadcast(128))
            avec_st = const.tile([128, 10], F32)
            nc.gpsimd.dma_start(out=avec_st, in_=avec_d[:])
            dvec = const.tile([128, 9], F32)
            nc.vector.tensor_copy(dvec, dvec_st)
            wobv_sb = const.tile([128, 384], F32)
            nc.vector.tensor_copy(wobv_sb, wobv_st)
            avec = const.tile([128, 10], F32)
            nc.scalar.activation(out=avec, in_=avec_st, func=AF.Copy)
            # Pre-load the combined exp+ln activation-table set (index 6,
            # natural_log_exp_and_others) so the table-load insertion pass
            # doesn't thrash between the exp-only and ln-only sets (31 loads
            # x 1.28us observed without this).
            nc.scalar.add_instruction(
                mybir.InstLoadActFuncSet(
                    name=nc.get_next_instruction_name(),
                    ins=[],
                    outs=[],
                    act_func_set_id=6,
                )
            )
            bqv_sb = dvec[:, 0:3]
            bkv_sb = dvec[:, 3:6]
            xtb_sb = dvec[:, 6:9]

            # ---- collective warmup: pay the first-call CC cost early -----
            wdum_in = dram.tile([128, 8], F32, name="wdum_in")
            wdum_out = dram.tile([128, 8], F32, name="wdum_out")
            nc.gpsimd.dma_start(out=wdum_in, in_=dvec_st[:, 0:8])
            nc.gpsimd.collective_compute(
                "AllReduce",
                ALU.add,
                replica_groups=GROUPS,
                ins=[wdum_in[:]],
                outs=[wdum_out[:]],
            )
            # (A second, full-size warmup AllReduce was tried here and made
            # things worse: its transfers coincided with a fully-throttled
            # 40us PE window mid-attention. One tiny warmup is enough to pay
            # the first-call collective cost.)

            # xt (attention output) slabs + h1 slabs, zero-padded 34x34
            xt_sl = []
            for i in range(3):
                t = xtp.tile([128, HP, HP], BF16, name=f"xt{i}")
                nc.vector.memset(t, 0.0)
                xt_sl.append(t)
            h1_sl = []
            for k in range(4):
                t = xtp.tile([128, HP, HP], BF16, name=f"h1{k}")
                nc.vector.memset(t, 0.0)
                h1_sl.append(t)
            # conv1 partial-sum accumulators (512 out ch as 4 m-tiles), bf16
            acc1 = [xtp.tile([128, S], BF16, name=f"acc1{m}") for m in range(4)]

            # Semaphore warmers: absorb const-DMA + memset waits into each
            # engine's observed clock so later compute ops need <=1 wait.
            warm = const.tile([128, 1], F32)
            nc.vector.tensor_copy(warm, dvec[:, 0:1])
            warm2 = const.tile([128, 1], F32)
            nc.scalar.activation(out=warm2, in_=warm, func=AF.Copy)

            # PE warm-up spam on zeroed slabs during the input-DMA window:
            # ~6us of continuous tiny matmuls flips HAM to K=8/8 before the
            # first projection matmul issues.
            wps = smps.tile([32, 512], F32, name="warmps", tag="sm")
            for w in range(40):
                nc.tensor.matmul(
                    wps[:, 0:32],
                    lhsT=xt_sl[0][:, 0, 0:32],
                    rhs=xt_sl[0][:, 1, 0:32],
                    start=True,
                    stop=True,
                )

            # k/q per branch (with biases added), uT tiles with 32 ones-cols:
            # PV stationary cols [0:32] = per-head u rows, cols [32:64] = 1.0
            # so the PV matmul lands numerator on PSUM rows 0:32 and the
            # softmax denominator broadcast on rows 32:64.
            k_sb = kq.tile([128, 3, S], BF16)
            q_sb = kq.tile([128, 3, S], BF16)
            uT = [kq.tile([128, 3, 4, 64], BF16, name=f"uT{t}") for t in range(8)]

            qsrc = [oth_sb[:, 0], x_sb, oth_sb[:, 1]]

            def kq_proj(i, which, s):
                """4 matmuls: half s of branch i's k or q projection.
                Allocated from smps, NOT scps: a kq tile in the score pool
                would consume the PE's 2-unit score lookahead during the
                branch-0 slots."""
                ps = smps.tile([128, 512], F32, name="kq_ps", tag="sm")
                w = wk_sb if which == "k" else wq_sb
                src = x_sb if which == "k" else qsrc[i]
                for ks in range(4):
                    nc.tensor.matmul(
                        ps,
                        lhsT=w[:, i, ks, :],
                        rhs=src[:, ks, 512 * s : 512 * (s + 1)],
                        start=(ks == 0),
                        stop=(ks == 3),
                    )
                dstt = k_sb if which == "k" else q_sb
                bias = bkv_sb if which == "k" else bqv_sb
                nc.vector.tensor_scalar_add(
                    dstt[:, i, 512 * s : 512 * (s + 1)], ps, bias[:, i : i + 1]
                )
                return 4

            def u_proj(t):
                """4 matmuls + uT assembly for token block t."""
                u_ps = smps.tile([128, 384], F32, name="u_ps", tag="sm")
                for ks in range(4):
                    nc.tensor.matmul(
                        u_ps,
                        lhsT=x_sb[:, ks, 128 * t : 128 * (t + 1)],
                        rhs=wvo_sb[:, ks, :],
                        start=(ks == 0),
                        stop=(ks == 3),
                    )
                nc.vector.memset(uT[t][:, :, :, 32:64], 1.0)
                nc.vector.tensor_add(
                    uT[t][:, :, :, 0:32],
                    u_ps.rearrange("p (i h d) -> p i h d", i=3, h=4),
                    wobv_sb.rearrange("p (i h d) -> p i h d", i=3, h=4),
                )
                return 4

            # conv weights (DMAs land during branch-0 attention)
            convw = ctx.enter_context(tc.tile_pool(name="convw", bufs=1))
            pt = ctx.enter_context(tc.tile_pool(name="pt", bufs=16))
            c1w_sb = [
                [convw.tile([128, 9, 128], BF16, name=f"c1w{i}_{m}") for m in range(4)]
                for i in range(3)
            ]
            c2w_sb = [convw.tile([128, 9, 128], BF16, name=f"c2w{k}") for k in range(4)]

            def emit_convw_dmas():
                for i in range(3):
                    for m in range(4):
                        nc.sync.dma_start(out=c1w_sb[i][m], in_=c1wT_d[i, m])
                for k in range(4):
                    nc.sync.dma_start(out=c2w_sb[k], in_=c2wT_d[k])

            partial = dram.tile([512, S], BF16, name="partial1")
            art = dram.tile([512, S], BF16, name="art")

            def conv1_block(i, m, n):
                """Partial conv1 for xt slab i, out m-tile, spatial half n,
                accumulated (bf16) into acc1[m]. 9 matmuls + 1 DVE op."""
                for piece in conv1_block_pieces(i, m, n):
                    piece()
                return 9

            def conv1_block_pieces(i, m, n):
                """The same block as three 3-matmul closures so the filler can
                spread it across units (keeps PE busy in ~0.6us grains and
                HAM at K=8/8 instead of idling between 1.9us lumps)."""
                state = {}

                def piece(dy):
                    def run():
                        if dy == 0:
                            state["ps"] = smps.tile([128, 512], F32, name="cv", tag="sm")
                        ps = state["ps"]
                        for dx in range(3):
                            nc.tensor.matmul(
                                ps,
                                lhsT=c1w_sb[i][m][:, dy * 3 + dx, :],
                                rhs=xt_sl[i][
                                    :, 16 * n + dy : 16 * n + dy + 16, dx : dx + 32
                                ],
                                start=(dy == 0 and dx == 0),
                                stop=(dy == 2 and dx == 2),
                            )
                        if dy == 2:
                            dst = acc1[m][:, 512 * n : 512 * (n + 1)]
                            if i == 0:
                                nc.vector.tensor_copy(dst, ps)
                            else:
                                nc.vector.tensor_add(dst, ps, dst)
                            if i == 2 and n == 1:
                                nc.sync.dma_start(
                                    out=partial[128 * m : 128 * (m + 1), :],
                                    in_=acc1[m],
                                )
                        return 3

                    return run

                return [piece(0), piece(1), piece(2)]

            # ---------------- attention pipeline --------------------------
            # 12 head-slots; slot emits its scores+exps unit by unit with the
            # previous slot's PV and filler matmuls interleaved.
            fillq = deque()
            for i in (1, 2):
                for s in range(2):
                    fillq.append(lambda i=i, s=s: kq_proj(i, "k", s))
                    fillq.append(lambda i=i, s=s: kq_proj(i, "q", s))

            fill_state = {"emitted": 0, "target": 0.0}

            def fill(budget):
                fill_state["target"] += budget
                while fillq and fill_state["emitted"] < fill_state["target"]:
                    fill_state["emitted"] += fillq.popleft()()

            pts = {}
            y_tiles = {}

            def emit_scores_unit(i, h, t):
                p0 = 32 * h
                sc = scps.tile([128, S], F32, name="sc", tag="sc")
                for s in range(2):
                    nc.tensor.matmul(
                        sc[:, 512 * s : 512 * (s + 1)],
                        lhsT=k_sb[p0 : p0 + 32, i, 128 * t : 128 * (t + 1)],
                        rhs=q_sb[p0 : p0 + 32, i, 512 * s : 512 * (s + 1)],
                        start=True,
                        stop=True,
                        tile_position=(p0, 0),
                    )
                ptt = pt.tile([128, S], BF16, name="ptt")
                nc.scalar.activation(out=ptt, in_=sc, func=AF.Exp, scale=float(ISQD))
                pts[(i, h, t)] = ptt

            def emit_pv_unit(i, h, u):
                s, j = u // 4, u % 4
                if j == 0:
                    # [128,512] occupies the same single PSUM bank as [64,512];
                    # rows 64:96 serve as a scratch target for the keep-warm
                    # spam matmuls (isolated per-element has_written bits).
                    y_tiles[(h % 2, s)] = smps.tile([128, 512], F32, name="y", tag="sm")
                y = y_tiles[(h % 2, s)]
                for tt in (2 * j, 2 * j + 1):
                    nc.tensor.matmul(
                        y[0:64, :],
                        lhsT=uT[tt][:, i, h, :],
                        rhs=pts[(i, h, tt)][:, 512 * s : 512 * (s + 1)],
                        start=(tt == 0),
                        stop=(tt == 7),
                    )

            lden_tiles = {}

            def norm_half(i, h, s):
                # 1/den via exp(-ln(den)) on ACT (DVE reciprocal is ~6.4ns/el
                # and HW rejects tensor-tensor divide), then DVE normalize +
                # write the xt slab. The s=0 half runs as soon as its PV
                # accumulation completes (mid-slot), spreading the ACT/DVE
                # load away from the slot boundary.
                p0 = 32 * h
                if s == 0:
                    lden_tiles[h % 2] = stg.tile([32, S], F32, name="lden")
                lden = lden_tiles[h % 2]
                y = y_tiles[(h % 2, s)]
                nc.scalar.activation(
                    out=lden[:, 512 * s : 512 * (s + 1)], in_=y[32:64, :], func=AF.Ln
                )
                rcp = stg.tile([32, 512], F32, name="rcp")
                nc.scalar.activation(
                    out=rcp, in_=lden[:, 512 * s : 512 * (s + 1)], func=AF.Exp,
                    scale=-1.0,
                )
                tmp = stg.tile([32, 512], F32, name="tmp")
                nc.vector.tensor_mul(tmp, y[0:32, :], rcp)
                nc.vector.tensor_scalar_add(
                    xt_sl[i][p0 : p0 + 32, 1 + 16 * s : 17 + 16 * s, 1:33],
                    tmp.rearrange("p (a b) -> p a b", b=32),
                    xtb_sb[p0 : p0 + 32, i : i + 1],
                )

            def finish_pv(i, h):
                norm_half(i, h, 1)
                for t in range(8):
                    del pts[(i, h, t)]

            # branch 0 k/q projections up front (scores need them)
            for s in range(2):
                kq_proj(0, "k", s)
                kq_proj(0, "q", s)
            emit_convw_dmas()

            slots = [(i, h) for i in range(3) for h in range(4)]
            prev = None
            pending_blocks = deque()  # (release_idx, closure)
            for idx, (i, h) in enumerate(slots):
                while pending_blocks and pending_blocks[0][0] <= idx:
                    fillq.append(pending_blocks.popleft()[1])
                for t in range(8):
                    emit_scores_unit(i, h, t)
                    if idx == 0:
                        # u-projections inline: uT[t] is always emitted well
                        # before the first PV unit that consumes it (slot 1).
                        u_proj(t)
                    if prev is not None:
                        emit_pv_unit(prev[0], prev[1], t)
                        if t == 3:
                            norm_half(prev[0], prev[1], 0)
                    # Front-load the slot's filler budget: the previous slot's
                    # Ln/Exp normalization chain sits ahead of this slot's
                    # exps in the ACT FIFO, so the PE needs ~2.3us of extra
                    # queued work right after the boundary to ride it out.
                    fill(5.0 if t < 2 else 4.0 / 3.0)
                if prev is not None:
                    finish_pv(prev[0], prev[1])
                    if prev[1] == 3:
                        # branch prev[0] xt complete once its finish_pv chain
                        # drains; release conv1 blocks one slot later so a
                        # block at the PE queue head never stalls on it.
                        for m in range(4):
                            for n in range(2):
                                for p in conv1_block_pieces(prev[0], m, n):
                                    pending_blocks.append((idx + 1, p))
                prev = (i, h)
            # drain: PV of the last slot; reserve ~2 filler pieces so the PE
            # has work while the last finish_pv chain runs on ACT/DVE
            for t in range(8):
                emit_pv_unit(prev[0], prev[1], t)
                if t == 3:
                    norm_half(prev[0], prev[1], 0)
                fill(1.5)
            finish_pv(prev[0], prev[1])
            while fillq:
                fillq.popleft()()
            # conv1 over branch 2's slab, m-major; AllReduce in two bf16
            # chunks so chunk 0 overlaps the m=2,3 conv1 work and conv2's
            # first half overlaps chunk 1.
            def ar_chunk(a):
                nc.gpsimd.collective_compute(
                    "AllReduce",
                    ALU.add,
                    replica_groups=GROUPS,
                    ins=[partial[256 * a : 256 * (a + 1), :]],
                    outs=[art[256 * a : 256 * (a + 1), :]],
                )

            for m in range(4):
                for n in range(2):
                    conv1_block(2, m, n)
                if m == 1:
                    ar_chunk(0)
            ar_chunk(1)

            arraw = stg.tile([128, 4, S], BF16, name="arraw", bufs=1)
            oout = stg.tile([128, S], F32, name="oout", bufs=1)
            ps2 = [smps.tile([128, 512], F32, name=f"cv2_{n}", tag="sm") for n in range(2)]

            for a in range(2):
                for k in (2 * a, 2 * a + 1):
                    nc.gpsimd.dma_start(
                        out=arraw[:, k, :],
                        in_=art[128 * k : 128 * (k + 1), :],
                    )
                for k in (2 * a, 2 * a + 1):
                    nc.scalar.activation(
                        out=h1_sl[k][:, 1:33, 1:33],
                        in_=arraw[:, k, :].rearrange("p (a b) -> p a b", b=32),
                        func=AF.Relu,
                        bias=avec[:, 4 + k : 5 + k],
                        scale=avec[:, k : k + 1],
                    )
                    # conv2 contribution of h1 slab k (both spatial halves)
                    for n in range(2):
                        for dy in range(3):
                            for dx in range(3):
                                nc.tensor.matmul(
                                    ps2[n],
                                    lhsT=c2w_sb[k][:, dy * 3 + dx, :],
                                    rhs=h1_sl[k][
                                        :, 16 * n + dy : 16 * n + dy + 16, dx : dx + 32
                                    ],
                                    start=(k == 0 and dy == 0 and dx == 0),
                                    stop=(k == 3 and dy == 2 and dx == 2),
                                )
            for n in range(2):
                nc.scalar.activation(
                    out=oout[:, 512 * n : 512 * (n + 1)],
                    in_=ps2[n],
                    func=AF.Relu,
                    bias=avec[:, 9:10],
                    scale=avec[:, 8:9],
                )
                nc.sync.dma_start(
                    out=out_d[:, 512 * n : 512 * (n + 1)],
                    in_=oout[:, 512 * n : 512 * (n + 1)],
                )

    nc.finalize()
    return nc


def _f(x):
    return np.ascontiguousarray(x, dtype=np.float32)


def _bf(x):
    return np.ascontiguousarray(np.asarray(x, dtype=np.float32).astype(ml_dtypes.bfloat16))


def prepare_core_inputs(inp):
    """Build the 8 per-core input dicts from the full-problem inputs."""
    inp = {k: np.asarray(v, dtype=np.float64) for k, v in inp.items()}
    x = inp["x"].reshape(B, C, S)
    xp = inp["x_prev"].reshape(B, C, S)
    xn = inp["x_next"].reshape(B, C, S)

    bn1s_full = inp["bn1g"] / np.sqrt(inp["bn1v"] + EPS)
    bn1b_full = inp["bn1b"] - inp["bn1m"] * bn1s_full
    bn2s_full = inp["bn2g"] / np.sqrt(inp["bn2v"] + EPS)
    bn2b_full = inp["bn2b"] - inp["bn2m"] * bn2s_full

    per_g = []
    for g in range(4):
        sl = slice(128 * g, 128 * (g + 1))
        wqT = np.stack(
            [
                np.stack([inp["Wq"][i][sl, 128 * k : 128 * (k + 1)].T for k in range(4)])
                for i in range(3)
            ]
        )
        wkT = np.stack(
            [
                np.stack([inp["Wk"][i][sl, 128 * k : 128 * (k + 1)].T for k in range(4)])
                for i in range(3)
            ]
        )
        bqv = np.stack([inp["bq"][i][sl] for i in range(3)], axis=1)
        bkv = np.stack([inp["bk"][i][sl] for i in range(3)], axis=1)

        att_s = np.stack(
            [inp["bng"][i][sl] / np.sqrt(inp["bnv"][i][sl] + EPS) for i in range(3)]
        )  # (3,128)
        xtb = np.stack(
            [
                inp["bnb"][i][sl] + (inp["bo"][i][sl] - inp["bnm"][i][sl]) * att_s[i]
                for i in range(3)
            ],
            axis=1,
        )  # (128,3)

        wvo_rows = []
        wobv_row = []
        for i in range(3):
            for hl in range(4):
                hg = 4 * g + hl
                wv_h = inp["Wv"][i][32 * hg : 32 * (hg + 1), :]  # (32, 512)
                bv_h = inp["bv"][i][32 * hg : 32 * (hg + 1)]
                wo_h = inp["Wo"][i, hg]  # (32, 32)
                sc = att_s[i][32 * hl : 32 * (hl + 1)]  # (32,)
                wvo_rows.append(sc[:, None] * (wo_h @ wv_h))
                wobv_row.append(sc * (wo_h @ bv_h))
        wvo_all = np.concatenate(wvo_rows, axis=0)  # (384, 512)
        wobv = np.concatenate(wobv_row)[None, :]  # (1, 384)
        wvoT = np.stack([wvo_all[:, 128 * k : 128 * (k + 1)].T for k in range(4)])

        c1wT = np.stack(
            [
                np.stack(
                    [
                        inp["c1w"][
                            128 * m : 128 * (m + 1),
                            512 * i + 128 * g : 512 * i + 128 * (g + 1),
                        ]
                        .transpose(1, 2, 3, 0)
                        .reshape(128, 9, 128)
                        for m in range(4)
                    ]
                )
                for i in range(3)
            ]
        )
        c2wT = np.stack(
            [
                inp["c2w"][sl, 128 * k : 128 * (k + 1)]
                .transpose(1, 2, 3, 0)
                .reshape(128, 9, 128)
                for k in range(4)
            ]
        )
        avec = np.concatenate(
            [
                bn1s_full.reshape(4, 128).T,
                bn1b_full.reshape(4, 128).T,
                bn2s_full[sl][:, None],
                bn2b_full[sl][:, None],
            ],
            axis=1,
        )  # (128, 10)

        per_g.append(
            dict(
                wqT=_bf(wqT), wkT=_bf(wkT), wvoT=_bf(wvoT),
                wobv=_f(wobv), c1wT=_bf(c1wT), c2wT=_bf(c2wT),
                dvec=_f(np.concatenate([bqv, bkv, xtb], axis=1)),
                avec=_f(avec),
            )
        )

    in_maps = []
    for c in range(NCORES):
        b, g = c // 4, c % 4
        d = dict(per_g[g])
        d["x4"] = _bf(x[b].reshape(4, 128, S))
        d["oth"] = _bf(np.stack([xn[b].reshape(4, 128, S), xp[b].reshape(4, 128, S)]))
        in_maps.append(d)
    return in_maps


_NC_CACHE = {}


def get_nc():
    if "nc" not in _NC_CACHE:
        _NC_CACHE["nc"] = build_nc()
    return _NC_CACHE["nc"]


def assemble(results):
    out = np.zeros((B, C, H, W), dtype=np.float32)
    for c in range(NCORES):
        b, g = c // 4, c % 4
        out[b, 128 * g : 128 * (g + 1)] = results[c]["out"].reshape(128, H, W)
    return out


def kernel(**inputs):
    nc = get_nc()
    in_maps = prepare_core_inputs(inputs)
    res = run_bass_kernel_spmd(nc, in_maps, list(range(NCORES)))
    return assemble(res.results)


# revision 42
# speedup vs baseline: 1.0030x; 1.0030x over previous
"""CSAEncoder Trainium2 kernel: 3-branch cross-attention + concat DoubleConv.

Sharding (8 cores): 2 batch groups x 4 tensor ranks.
Core c: batch b = c // 4, rank g = c % 4.
  - Attention: core computes heads [4g, 4g+4) of all 3 branches for batch b
    (a contiguous 128-channel slab of each branch's output).
  - conv1 computed as partial sums over the core's local 384 input channels
    for ALL 512 output channels; one bf16 AllReduce(add) within the 4-core
    batch group gives every rank the full conv1 pre-BN output.
  - conv2 computed locally: full 512-channel contraction, only the core's own
    128 output channels. No second collective.
Host assembles the full (2, 512, 32, 32) output from the 8 per-core slabs.

v2 schedule: the attention is emitted as a flat pipeline of 12 head-slots
(branch x head). Each slot emits its 16 score matmuls unit-by-unit with the
previous slot's PV matmuls and filler work (projections for later branches,
conv1 blocks of finished branches) interleaved, so the in-order PE queue never
blocks on the scalar engine's exp stream and HAM stays at K=8/8. The softmax
denominator is produced broadcast on PSUM partitions 32:64 via 32 ones-columns
in the PV stationary, so normalization is a single DVE divide (no reciprocal,
no DRAM round-trip). conv1 partials accumulate in bf16 and are AllReduced in
bf16 as one chunk, with a tiny warmup collective issued at kernel start to pay
the first-call collective cost off the critical path.
"""

import os
import sys

import ml_dtypes
import numpy as np

for _p in ("/opt/trn_rl_repo",):
    if _p not in sys.path and os.path.isdir(_p):
        sys.path.insert(0, _p)

import concourse.bass as bass
import concourse.mybir as mybir
import concourse.tile as tile
from concourse import bacc
from concourse.bass_utils import run_bass_kernel_spmd

F32 = mybir.dt.float32
BF16 = mybir.dt.bfloat16
AF = mybir.ActivationFunctionType
ALU = mybir.AluOpType

B, C, H, W, HEADS = 2, 512, 32, 32, 16
D = C // HEADS            # 32
S = H * W                 # 1024
EPS = 1e-5
ISQD = 1.0 / np.sqrt(D)   # folded into the exp activation
NCORES = 8
GROUPS = [[0, 1, 2, 3], [4, 5, 6, 7]]
HP = W + 2                # padded row stride (34)


def build_nc():
    nc = bacc.Bacc(None, target_bir_lowering=False)

    # ---- per-core external inputs -------------------------------------
    x4_d = nc.declare_dram_parameter("x4", [4, 128, S], BF16, isOutput=False)
    oth_d = nc.declare_dram_parameter("oth", [2, 4, 128, S], BF16, isOutput=False)
    wqT_d = nc.declare_dram_parameter("wqT", [3, 4, 128, 128], BF16, isOutput=False)
    wkT_d = nc.declare_dram_parameter("wkT", [3, 4, 128, 128], BF16, isOutput=False)
    wvoT_d = nc.declare_dram_parameter("wvoT", [4, 128, 384], BF16, isOutput=False)
    dvec_d = nc.declare_dram_parameter("dvec", [128, 9], F32, isOutput=False)
    wobv_d = nc.declare_dram_parameter("wobv", [1, 384], F32, isOutput=False)
    c1wT_d = nc.declare_dram_parameter("c1wT", [3, 4, 128, 9, 128], BF16, isOutput=False)
    c2wT_d = nc.declare_dram_parameter("c2wT", [4, 128, 9, 128], BF16, isOutput=False)
    avec_d = nc.declare_dram_parameter("avec", [128, 10], F32, isOutput=False)
    out_d = nc.declare_dram_parameter("out", [128, S], F32, isOutput=True)

    with tile.TileContext(nc) as tc:
        import contextlib
        from collections import deque

        ctx = contextlib.ExitStack()
        with ctx:
            const = ctx.enter_context(tc.tile_pool(name="const", bufs=1))
            kq = ctx.enter_context(tc.tile_pool(name="kq", bufs=1))
            xtp = ctx.enter_context(tc.tile_pool(name="xtp", bufs=1))
            stg = ctx.enter_context(tc.tile_pool(name="stg", bufs=4))
            scps = ctx.enter_context(tc.tile_pool(name="scps", bufs=2, space="PSUM"))
            smps = ctx.enter_context(tc.tile_pool(name="smps", bufs=4, space="PSUM"))
            dram = ctx.enter_context(tc.tile_pool(name="dram", bufs=1, space="DRAM"))

            # ---- activations first (highest DMA priority) ----------------
            x_sb = const.tile([128, 4, S], BF16)
            oth_sb = const.tile([128, 2, 4, S], BF16)
            wq_sb = const.tile([128, 3, 4, 128], BF16)
            wk_sb = const.tile([128, 3, 4, 128], BF16)
            wvo_sb = const.tile([128, 4, 384], BF16)
            # DMA order = need order: x + branch-0 weights + q-source (oth0)
            # gate the first scores; everything else lands under branch 0.
            for ks in range(4):
                nc.sync.dma_start(out=x_sb[:, ks, :], in_=x4_d[ks])
            for ks in range(4):
                nc.sync.dma_start(out=wk_sb[:, 0, ks, :], in_=wkT_d[0, ks])
            for ks in range(4):
                nc.sync.dma_start(out=oth_sb[:, 0, ks, :], in_=oth_d[0, ks])
            for ks in range(4):
                nc.sync.dma_start(out=wq_sb[:, 0, ks, :], in_=wqT_d[0, ks])
            for ks in range(4):
                nc.sync.dma_start(out=wvo_sb[:, ks, :], in_=wvoT_d[ks])
            for i in (1, 2):
                for ks in range(4):
                    nc.sync.dma_start(out=wk_sb[:, i, ks, :], in_=wkT_d[i, ks])
                    nc.sync.dma_start(out=wq_sb[:, i, ks, :], in_=wqT_d[i, ks])
            for ks in range(4):
                nc.sync.dma_start(out=oth_sb[:, 1, ks, :], in_=oth_d[1, ks])

            # Small consts: DMA to staging, then re-own on the consuming
            # engine (DVE / ACT) so consumers need no cross-engine const wait.
            dvec_st = const.tile([128, 9], F32)
            nc.gpsimd.dma_start(out=dvec_st, in_=dvec_d[:])
            wobv_st = const.tile([128, 384], F32)
            nc.gpsimd.dma_start(out=wobv_st, in_=wobv_d[:].partition_broadcast(128))
            avec_st = const.tile([128, 10], F32)
            nc.gpsimd.dma_start(out=avec_st, in_=avec_d[:])
            dvec = const.tile([128, 9], F32)
            nc.vector.tensor_copy(dvec, dvec_st)
            wobv_sb = const.tile([128, 384], F32)
            nc.vector.tensor_copy(wobv_sb, wobv_st)
            avec = const.tile([128, 10], F32)
            nc.scalar.activation(out=avec, in_=avec_st, func=AF.Copy)
            # Pre-load the combined exp+ln activation-table set (index 6,
            # natural_log_exp_and_others) so the table-load insertion pass
            # doesn't thrash between the exp-only and ln-only sets (31 loads
            # x 1.28us observed without this).
            nc.scalar.add_instruction(
                mybir.InstLoadActFuncSet(
                    name=nc.get_next_instruction_name(),
                    ins=[],
                    outs=[],
                    act_func_set_id=6,
                )
            )
            bqv_sb = dvec[:, 0:3]
            bkv_sb = dvec[:, 3:6]
            xtb_sb = dvec[:, 6:9]

            # ---- collective warmup: pay the first-call CC cost early -----
            wdum_in = dram.tile([128, 8], F32, name="wdum_in")
            wdum_out = dram.tile([128, 8], F32, name="wdum_out")
            nc.gpsimd.dma_start(out=wdum_in, in_=dvec_st[:, 0:8])
            nc.gpsimd.collective_compute(
                "AllReduce",
                ALU.add,
                replica_groups=GROUPS,
                ins=[wdum_in[:]],
                outs=[wdum_out[:]],
            )
            # (A second, full-size warmup AllReduce was tried here and made
            # things worse: its transfers coincided with a fully-throttled
            # 40us PE window mid-attention. One tiny warmup is enough to pay
            # the first-call collective cost.)

            # xt (attention output) slabs + h1 slabs, zero-padded 34x34
            xt_sl = []
            for i in range(3):
                t = xtp.tile([128, HP, HP], BF16, name=f"xt{i}")
                nc.vector.memset(t, 0.0)
                xt_sl.append(t)
            h1_sl = []
            for k in range(4):
                t = xtp.tile([128, HP, HP], BF16, name=f"h1{k}")
                nc.vector.memset(t, 0.0)
                h1_sl.append(t)
            # conv1 partial-sum accumulators (512 out ch as 4 m-tiles), bf16
            acc1 = [xtp.tile([128, S], BF16, name=f"acc1{m}") for m in range(4)]

            # Semaphore warmers: absorb const-DMA + memset waits into each
            # engine's observed clock so later compute ops need <=1 wait.
            warm = const.tile([128, 1], F32)
            nc.vector.tensor_copy(warm, dvec[:, 0:1])
            warm2 = const.tile([128, 1], F32)
            nc.scalar.activation(out=warm2, in_=warm, func=AF.Copy)

            # PE warm-up spam on zeroed slabs during the input-DMA window:
            # ~6us of continuous tiny matmuls flips HAM to K=8/8 before the
            # first projection matmul issues.
            wps = smps.tile([32, 512], F32, name="warmps", tag="sm")
            for w in range(40):
                nc.tensor.matmul(
                    wps[:, 0:32],
                    lhsT=xt_sl[0][:, 0, 0:32],
                    rhs=xt_sl[0][:, 1, 0:32],
                    start=True,
                    stop=True,
                )

            # k/q per branch (with biases added), uT tiles with 32 ones-cols:
            # PV stationary cols [0:32] = per-head u rows, cols [32:64] = 1.0
            # so the PV matmul lands numerator on PSUM rows 0:32 and the
            # softmax denominator broadcast on rows 32:64.
            k_sb = kq.tile([128, 3, S], BF16)
            q_sb = kq.tile([128, 3, S], BF16)
            uT = [kq.tile([128, 3, 4, 64], BF16, name=f"uT{t}") for t in range(8)]

            qsrc = [oth_sb[:, 0], x_sb, oth_sb[:, 1]]

            def kq_proj(i, which, s):
                """4 matmuls: half s of branch i's k or q projection.
                Allocated from smps, NOT scps: a kq tile in the score pool
                would consume the PE's 2-unit score lookahead during the
                branch-0 slots."""
                ps = smps.tile([128, 512], F32, name="kq_ps", tag="sm")
                w = wk_sb if which == "k" else wq_sb
                src = x_sb if which == "k" else qsrc[i]
                for ks in range(4):
                    nc.tensor.matmul(
                        ps,
                        lhsT=w[:, i, ks, :],
                        rhs=src[:, ks, 512 * s : 512 * (s + 1)],
                        start=(ks == 0),
                        stop=(ks == 3),
                    )
                dstt = k_sb if which == "k" else q_sb
                bias = bkv_sb if which == "k" else bqv_sb
                nc.vector.tensor_scalar_add(
                    dstt[:, i, 512 * s : 512 * (s + 1)], ps, bias[:, i : i + 1]
                )
                return 4

            def u_proj(t):
                """4 matmuls + uT assembly for token block t."""
                u_ps = smps.tile([128, 384], F32, name="u_ps", tag="sm")
                for ks in range(4):
                    nc.tensor.matmul(
                        u_ps,
                        lhsT=x_sb[:, ks, 128 * t : 128 * (t + 1)],
                        rhs=wvo_sb[:, ks, :],
                        start=(ks == 0),
                        stop=(ks == 3),
                    )
                nc.vector.memset(uT[t][:, :, :, 32:64], 1.0)
                nc.vector.tensor_add(
                    uT[t][:, :, :, 0:32],
                    u_ps.rearrange("p (i h d) -> p i h d", i=3, h=4),
                    wobv_sb.rearrange("p (i h d) -> p i h d", i=3, h=4),
                )
                return 4

            # conv weights (DMAs land during branch-0 attention)
            convw = ctx.enter_context(tc.tile_pool(name="convw", bufs=1))
            pt = ctx.enter_context(tc.tile_pool(name="pt", bufs=16))
            c1w_sb = [
                [convw.tile([128, 9, 128], BF16, name=f"c1w{i}_{m}") for m in range(4)]
                for i in range(3)
            ]
            c2w_sb = [convw.tile([128, 9, 128], BF16, name=f"c2w{k}") for k in range(4)]

            def emit_convw_dmas():
                for i in range(3):
                    for m in range(4):
                        nc.sync.dma_start(out=c1w_sb[i][m], in_=c1wT_d[i, m])
                for k in range(4):
                    nc.sync.dma_start(out=c2w_sb[k], in_=c2wT_d[k])

            partial = dram.tile([512, S], BF16, name="partial1")
            art = dram.tile([512, S], BF16, name="art")

            def conv1_block(i, m, n):
                """Partial conv1 for xt slab i, out m-tile, spatial half n,
                accumulated (bf16) into acc1[m]. 9 matmuls + 1 DVE op."""
                for piece in conv1_block_pieces(i, m, n):
                    piece()
                return 9

            def conv1_block_pieces(i, m, n):
                """The same block as three 3-matmul closures so the filler can
                spread it across units (keeps PE busy in ~0.6us grains and
                HAM at K=8/8 instead of idling between 1.9us lumps)."""
                state = {}

                def piece(dy):
                    def run():
                        if dy == 0:
                            state["ps"] = smps.tile([128, 512], F32, name="cv", tag="sm")
                        ps = state["ps"]
                        for dx in range(3):
                            nc.tensor.matmul(
                                ps,
                                lhsT=c1w_sb[i][m][:, dy * 3 + dx, :],
                                rhs=xt_sl[i][
                                    :, 16 * n + dy : 16 * n + dy + 16, dx : dx + 32
                                ],
                                start=(dy == 0 and dx == 0),
                                stop=(dy == 2 and dx == 2),
                            )
                        if dy == 2:
                            dst = acc1[m][:, 512 * n : 512 * (n + 1)]
                            if i == 0:
                                nc.vector.tensor_copy(dst, ps)
                            else:
                                nc.vector.tensor_add(dst, ps, dst)
                            if i == 2 and n == 1:
                                nc.sync.dma_start(
                                    out=partial[128 * m : 128 * (m + 1), :],
                                    in_=acc1[m],
                                )
                        return 3

                    return run

                return [piece(0), piece(1), piece(2)]

            # ---------------- attention pipeline --------------------------
            # 12 head-slots; slot emits its scores+exps unit by unit with the
            # previous slot's PV and filler matmuls interleaved.
            fillq = deque()
            for i in (1, 2):
                for s in range(2):
                    fillq.append(lambda i=i, s=s: kq_proj(i, "k", s))
                    fillq.append(lambda i=i, s=s: kq_proj(i, "q", s))

            fill_state = {"emitted": 0, "target": 0.0}

            def fill(budget):
                fill_state["target"] += budget
                while fillq and fill_state["emitted"] < fill_state["target"]:
                    fill_state["emitted"] += fillq.popleft()()

            pts = {}
            y_tiles = {}

            def emit_scores_unit(i, h, t):
                p0 = 32 * h
                sc = scps.tile([128, S], F32, name="sc", tag="sc")
                for s in range(2):
                    nc.tensor.matmul(
                        sc[:, 512 * s : 512 * (s + 1)],
                        lhsT=k_sb[p0 : p0 + 32, i, 128 * t : 128 * (t + 1)],
                        rhs=q_sb[p0 : p0 + 32, i, 512 * s : 512 * (s + 1)],
                        start=True,
                        stop=True,
                        tile_position=(p0, 0),
                    )
                ptt = pt.tile([128, S], BF16, name="ptt")
                nc.scalar.activation(out=ptt, in_=sc, func=AF.Exp, scale=float(ISQD))
                pts[(i, h, t)] = ptt

            def emit_pv_unit(i, h, u):
                s, j = u // 4, u % 4
                if j == 0:
                    # [128,512] occupies the same single PSUM bank as [64,512];
                    # rows 64:96 serve as a scratch target for the keep-warm
                    # spam matmuls (isolated per-element has_written bits).
                    y_tiles[(h % 2, s)] = smps.tile([128, 512], F32, name="y", tag="sm")
                y = y_tiles[(h % 2, s)]
                for tt in (2 * j, 2 * j + 1):
                    nc.tensor.matmul(
                        y[0:64, :],
                        lhsT=uT[tt][:, i, h, :],
                        rhs=pts[(i, h, tt)][:, 512 * s : 512 * (s + 1)],
                        start=(tt == 0),
                        stop=(tt == 7),
                    )

            lden_tiles = {}

            def norm_half(i, h, s):
                # 1/den via exp(-ln(den)) on ACT (DVE reciprocal is ~6.4ns/el
                # and HW rejects tensor-tensor divide), then DVE normalize +
                # write the xt slab. The s=0 half runs as soon as its PV
                # accumulation completes (mid-slot), spreading the ACT/DVE
                # load away from the slot boundary.
                p0 = 32 * h
                if s == 0:
                    lden_tiles[h % 2] = stg.tile([32, S], F32, name="lden")
                lden = lden_tiles[h % 2]
                y = y_tiles[(h % 2, s)]
                nc.scalar.activation(
                    out=lden[:, 512 * s : 512 * (s + 1)], in_=y[32:64, :], func=AF.Ln
                )
                rcp = stg.tile([32, 512], F32, name="rcp")
                nc.scalar.activation(
                    out=rcp, in_=lden[:, 512 * s : 512 * (s + 1)], func=AF.Exp,
                    scale=-1.0,
                )
                tmp = stg.tile([32, 512], F32, name="tmp")
                nc.vector.tensor_mul(tmp, y[0:32, :], rcp)
                nc.vector.tensor_scalar_add(
                    xt_sl[i][p0 : p0 + 32, 1 + 16 * s : 17 + 16 * s, 1:33],
                    tmp.rearrange("p (a b) -> p a b", b=32),
                    xtb_sb[p0 : p0 + 32, i : i + 1],
                )

            def finish_pv(i, h):
                norm_half(i, h, 1)
                for t in range(8):
                    del pts[(i, h, t)]

            # branch 0 k/q projections up front (scores need them)
            for s in range(2):
                kq_proj(0, "k", s)
                kq_proj(0, "q", s)
            emit_convw_dmas()

            slots = [(i, h) for i in range(3) for h in range(4)]
            prev = None
            pending_blocks = deque()  # (release_idx, closure)
            for idx, (i, h) in enumerate(slots):
                while pending_blocks and pending_blocks[0][0] <= idx:
                    fillq.append(pending_blocks.popleft()[1])
                for t in range(8):
                    emit_scores_unit(i, h, t)
                    if idx == 0:
                        # u-projections inline: uT[t] is always emitted well
                        # before the first PV unit that consumes it (slot 1).
                        u_proj(t)
                    if prev is not None:
                        emit_pv_unit(prev[0], prev[1], t)
                        if t == 3:
                            norm_half(prev[0], prev[1], 0)
                    # Front-load the slot's filler budget: the previous slot's
                    # Ln/Exp normalization chain sits ahead of this slot's
                    # exps in the ACT FIFO, so the PE needs ~2.3us of extra
                    # queued work right after the boundary to ride it out.
                    fill(5.0 if t < 2 else 4.0 / 3.0)
                if prev is not None:
                    finish_pv(prev[0], prev[1])
                    if prev[1] == 3:
                        # branch prev[0] xt complete once its finish_pv chain
                        # drains; release conv1 blocks one slot later so a
                        # block at the PE queue head never stalls on it.
                        for m in range(4):
                            for n in range(2):
                                for p in conv1_block_pieces(prev[0], m, n):
                                    pending_blocks.append((idx + 1, p))
                prev = (i, h)
            # drain: PV of the last slot; reserve ~2 filler pieces so the PE
            # has work while the last finish_pv chain runs on ACT/DVE
            for t in range(8):
                emit_pv_unit(prev[0], prev[1], t)
                if t == 3:
                    norm_half(prev[0], prev[1], 0)
                fill(1.5)
            finish_pv(prev[0], prev[1])
            while fillq:
                fillq.popleft()()
            # conv1 over branch 2's slab, m-major; AllReduce in two bf16
            # chunks so chunk 0 overlaps the m=2,3 conv1 work and conv2's
            # first half overlaps chunk 1.
            def ar_chunk(lo, hi):
                nc.gpsimd.collective_compute(
                    "AllReduce",
                    ALU.add,
                    replica_groups=GROUPS,
                    ins=[partial[128 * lo : 128 * hi, :]],
                    outs=[art[128 * lo : 128 * hi, :]],
                )

            # Asymmetric chunks: m0 alone ships as soon as its conv1 finishes
            # (both chunks are near the mesh latency floor anyway), so conv2's
            # k=0 slab starts while the m1-m3 chunk is still in flight.
            for m in range(4):
                for n in range(2):
                    conv1_block(2, m, n)
                if m == 0:
                    ar_chunk(0, 1)
            ar_chunk(1, 4)

            arraw = stg.tile([128, 4, S], BF16, name="arraw", bufs=1)
            oout = stg.tile([128, S], F32, name="oout", bufs=1)
            ps2 = [smps.tile([128, 512], F32, name=f"cv2_{n}", tag="sm") for n in range(2)]

            for ks_group in ((0,), (1, 2, 3)):
                for k in ks_group:
                    nc.gpsimd.dma_start(
                        out=arraw[:, k, :],
                        in_=art[128 * k : 128 * (k + 1), :],
                    )
                for k in ks_group:
                    nc.scalar.activation(
                        out=h1_sl[k][:, 1:33, 1:33],
                        in_=arraw[:, k, :].rearrange("p (a b) -> p a b", b=32),
                        func=AF.Relu,
                        bias=avec[:, 4 + k : 5 + k],
                        scale=avec[:, k : k + 1],
                    )
                    # conv2 contribution of h1 slab k (both spatial halves)
                    for n in range(2):
                        for dy in range(3):
                            for dx in range(3):
                                nc.tensor.matmul(
                                    ps2[n],
                                    lhsT=c2w_sb[k][:, dy * 3 + dx, :],
                                    rhs=h1_sl[k][
                                        :, 16 * n + dy : 16 * n + dy + 16, dx : dx + 32
                                    ],
                                    start=(k == 0 and dy == 0 and dx == 0),
                                    stop=(k == 3 and dy == 2 and dx == 2),
                                )
            for n in range(2):
                nc.scalar.activation(
                    out=oout[:, 512 * n : 512 * (n + 1)],
                    in_=ps2[n],
                    func=AF.Relu,
                    bias=avec[:, 9:10],
                    scale=avec[:, 8:9],
                )
                nc.sync.dma_start(
                    out=out_d[:, 512 * n : 512 * (n + 1)],
                    in_=oout[:, 512 * n : 512 * (n + 1)],
                )

    nc.finalize()
    return nc


def _f(x):
    return np.ascontiguousarray(x, dtype=np.float32)


def _bf(x):
    return np.ascontiguousarray(np.asarray(x, dtype=np.float32).astype(ml_dtypes.bfloat16))


def prepare_core_inputs(inp):
    """Build the 8 per-core input dicts from the full-problem inputs."""
    inp = {k: np.asarray(v, dtype=np.float64) for k, v in inp.items()}
    x = inp["x"].reshape(B, C, S)
    xp = inp["x_prev"].reshape(B, C, S)
    xn = inp["x_next"].reshape(B, C, S)

    bn1s_full = inp["bn1g"] / np.sqrt(inp["bn1v"] + EPS)
    bn1b_full = inp["bn1b"] - inp["bn1m"] * bn1s_full
    bn2s_full = inp["bn2g"] / np.sqrt(inp["bn2v"] + EPS)
    bn2b_full = inp["bn2b"] - inp["bn2m"] * bn2s_full

    per_g = []
    for g in range(4):
        sl = slice(128 * g, 128 * (g + 1))
        wqT = np.stack(
            [
                np.stack([inp["Wq"][i][sl, 128 * k : 128 * (k + 1)].T for k in range(4)])
                for i in range(3)
            ]
        )
        wkT = np.stack(
            [
                np.stack([inp["Wk"][i][sl, 128 * k : 128 * (k + 1)].T for k in range(4)])
                for i in range(3)
            ]
        )
        bqv = np.stack([inp["bq"][i][sl] for i in range(3)], axis=1)
        bkv = np.stack([inp["bk"][i][sl] for i in range(3)], axis=1)

        att_s = np.stack(
            [inp["bng"][i][sl] / np.sqrt(inp["bnv"][i][sl] + EPS) for i in range(3)]
        )  # (3,128)
        xtb = np.stack(
            [
                inp["bnb"][i][sl] + (inp["bo"][i][sl] - inp["bnm"][i][sl]) * att_s[i]
                for i in range(3)
            ],
            axis=1,
        )  # (128,3)

        wvo_rows = []
        wobv_row = []
        for i in range(3):
            for hl in range(4):
                hg = 4 * g + hl
                wv_h = inp["Wv"][i][32 * hg : 32 * (hg + 1), :]  # (32, 512)
                bv_h = inp["bv"][i][32 * hg : 32 * (hg + 1)]
                wo_h = inp["Wo"][i, hg]  # (32, 32)
                sc = att_s[i][32 * hl : 32 * (hl + 1)]  # (32,)
                wvo_rows.append(sc[:, None] * (wo_h @ wv_h))
                wobv_row.append(sc * (wo_h @ bv_h))
        wvo_all = np.concatenate(wvo_rows, axis=0)  # (384, 512)
        wobv = np.concatenate(wobv_row)[None, :]  # (1, 384)
        wvoT = np.stack([wvo_all[:, 128 * k : 128 * (k + 1)].T for k in range(4)])

        c1wT = np.stack(
            [
                np.stack(
                    [
                        inp["c1w"][
                            128 * m : 128 * (m + 1),
                            512 * i + 128 * g : 512 * i + 128 * (g + 1),
                        ]
                        .transpose(1, 2, 3, 0)
                        .reshape(128, 9, 128)
                        for m in range(4)
                    ]
                )
                for i in range(3)
            ]
        )
        c2wT = np.stack(
            [
                inp["c2w"][sl, 128 * k : 128 * (k + 1)]
                .transpose(1, 2, 3, 0)
                .reshape(128, 9, 128)
                for k in range(4)
            ]
        )
        avec = np.concatenate(
            [
                bn1s_full.reshape(4, 128).T,
                bn1b_full.reshape(4, 128).T,
                bn2s_full[sl][:, None],
                bn2b_full[sl][:, None],
            ],
            axis=1,
        )  # (128, 10)

        per_g.append(
            dict(
                wqT=_bf(wqT), wkT=_bf(wkT), wvoT=_bf(wvoT),
                wobv=_f(wobv), c1wT=_bf(c1wT), c2wT=_bf(c2wT),
                dvec=_f(np.concatenate([bqv, bkv, xtb], axis=1)),
                avec=_f(avec),
            )
        )

    in_maps = []
    for c in range(NCORES):
        b, g = c // 4, c % 4
        d = dict(per_g[g])
        d["x4"] = _bf(x[b].reshape(4, 128, S))
        d["oth"] = _bf(np.stack([xn[b].reshape(4, 128, S), xp[b].reshape(4, 128, S)]))
        in_maps.append(d)
    return in_maps


_NC_CACHE = {}


def get_nc():
    if "nc" not in _NC_CACHE:
        _NC_CACHE["nc"] = build_nc()
    return _NC_CACHE["nc"]


def assemble(results):
    out = np.zeros((B, C, H, W), dtype=np.float32)
    for c in range(NCORES):
        b, g = c // 4, c % 4
        out[b, 128 * g : 128 * (g + 1)] = results[c]["out"].reshape(128, H, W)
    return out


def kernel(**inputs):
    nc = get_nc()
    in_maps = prepare_core_inputs(inputs)
    res = run_bass_kernel_spmd(nc, in_maps, list(range(NCORES)))
    return assemble(res.results)
